# revision 22
# baseline (speedup 1.0000x reference)
"""DESimplE scoring kernel v3 for 8 Trainium2 NeuronCores.

v2 (fp16 mega-table + per-128-row indirect DMAs) ran at ~869 us with
GpSimd descriptor generation as the top serialized engine (512 x 1.3 us)
and DVE at ~530 us. v3 restructures around batched SWDGE gathers and an
int8-quantized table:

  * dma_gather (InstDMAGatherAnt): one instruction gathers hundreds of
    rows (994 ns fixed + 0.34 ns/row) instead of one 128-row indirect
    DMA per 1.3 us. Its int16 row index limits the addressable table to
    32K rows, so the 200K-entity table is split into 7 banks and the
    batch is bucket-sorted host-side by (s_bank, o_bank); each bucket
    chunk is gathered from a single bank base.
  * Rows are 1024 B: e_s|e_o kept fp16 (256 B), frq/phi/amp quantized
    int8 (216 B each, scale QS = 0.2/127, clamped at 4 sigma), 120 B pad
    (dma_gather requires elem_size % 256 == 0). Halves gather bytes.
  * DVE ops read int8 operands directly (mixed-dtype tensor_tensor);
    quant scales are folded into the relation matrix, the activation
    scale of Sin, and one scalar_tensor_tensor.
  * U' = frq_q*t + phi_q is computed in "quant units" so the phi add is
    a plain tensor_tensor; Sin applies the QS scale. |frq| <= 0.2
    guarantees |U| < pi for the year/month planes; only the day plane
    needs the round-to-i16 range reduction.
  * Final 200-column reduction is one DVE tensor_reduce into fp32.
  * The batch is dealt bucket-evenly across cores so the compiled
    program (gather run structure) is identical for all 8 cores (SPMD);
    bucket chunks are padded to 128-slot tiles with real index 0 pads
    (~+10-15% slots), dropped on the host.

Score layout per slot column (Z, 200 fp16):
  [0:64)    es(s) * 256*rf64 * eo(o)
  [64:100)  (Ts(s)*QS) * 256*QS*rf36 * To(o)
  [100:164) eo(s) * 256*ri64 * es(o)
  [164:200) (To(s)*QS) * 256*QS*ri36 * Ts(o)
Host divides the accumulated output by 512 (256 = 0.5*512 folded into
the relation columns).
"""

import numpy as np
from contextlib import ExitStack

import concourse.bass as bass
import concourse.bacc as bacc
import concourse.tile as tile
from concourse import mybir
from concourse.alu_op_type import AluOpType
from concourse.bass_utils import run_bass_kernel_spmd

NE, NR, B = 200000, 500, 262144
S_DIM, T_DIM = 64, 36
NCORES = 8
P = 128
BC = B // NCORES            # 32768 elements per core
NBANK = 7
BANKR = 28672               # rows per bank; 7*28672 = 200704 >= NE
NE_PAD = NBANK * BANKR
ROWB = 1024                 # bytes per table row
OFF_FRQ, OFF_PHI, OFF_AMP = 256, 472, 688
QS = 0.2 / 127.0            # int8 quant scale for frq/phi/amp
RROW = 200
KOUT = 512.0
G = 12                      # tiles per pipeline group
TWO_PI = float(2 * np.pi)
TADD_GPS = False            # GPS TT is in a different ucode library than
                            # dma_gather -> per-group library reloads; keep off
FOLD_GPS = False            # GPS tensor_reduce is partition-axis only

F32 = mybir.dt.float32
F16 = mybir.dt.float16
I16 = mybir.dt.int16
I8 = mybir.dt.int8


# ----------------------------------------------------------------------------
# device program
# ----------------------------------------------------------------------------

def build_nc(nt, runs):
    """nt: total tiles per core. runs: list of
    (side, bank, slot_start, nrows) with slot_start/nrows multiples of
    128, pre-split at group boundaries, identical across cores."""
    nc = bacc.Bacc(num_swdge_queues=4)

    tbl = nc.declare_dram_parameter("tbl", [NE_PAD, ROWB], I8, isOutput=False)
    # idx streams are wrapped in 16 partitions and replicated 8x (one copy
    # per GPSIMD core's partition group)
    sidx = nc.declare_dram_parameter("sidx", [P, nt * 8], I16, isOutput=False)
    oidx = nc.declare_dram_parameter("oidx", [P, nt * 8], I16, isOutput=False)
    rmat = nc.declare_dram_parameter("rmat", [P, nt * RROW], F16, isOutput=False)
    tv = nc.declare_dram_parameter("tv", [P, nt * 3], F16, isOutput=False)
    out = nc.declare_dram_parameter("out", [P, nt], F32, isOutput=True)

    ngroups = (nt + G - 1) // G
    # bin runs by group
    runs_by_group = [[] for _ in range(ngroups)]
    for side, bank, start, nrows in runs:
        g = start // (G * P)
        assert (start + nrows - 1) // (G * P) == g
        runs_by_group[g].append((side, bank, start, nrows))

    with ExitStack() as ctx:
        tc = ctx.enter_context(tile.TileContext(nc))
        cpool = ctx.enter_context(tc.tile_pool(name="const", bufs=1))
        mpool = ctx.enter_context(tc.tile_pool(name="m", bufs=3))
        rpool = ctx.enter_context(tc.tile_pool(name="r", bufs=2))
        upool = ctx.enter_context(tc.tile_pool(name="u", bufs=2))
        ppool = ctx.enter_context(tc.tile_pool(name="ph", bufs=2))
        qpool = ctx.enter_context(tc.tile_pool(name="q", bufs=2))
        spool = ctx.enter_context(tc.tile_pool(name="s", bufs=2))
        tpool = ctx.enter_context(tc.tile_pool(name="t", bufs=2))
        zpool = ctx.enter_context(tc.tile_pool(name="z", bufs=2))

        sidx_t = cpool.tile([P, nt * 8], I16)
        nc.sync.dma_start(sidx_t[:], sidx[:, :])
        oidx_t = cpool.tile([P, nt * 8], I16)
        nc.sync.dma_start(oidx_t[:], oidx[:, :])
        tv_t = cpool.tile([P, nt * 3], F16)
        nc.sync.dma_start(tv_t[:], tv[:, :])
        oacc = cpool.tile([P, nt], F32)
        idx_tiles = {0: sidx_t, 1: oidx_t}

        qn = [0]
        st = {}  # per-group live tiles

        def phase1(g):
            """gathers + R load + ACT casts for group g."""
            t0 = g * G
            gn = min(G, nt - t0)
            M = mpool.tile([P, 2 * gn * ROWB], I8)
            for side, bank, start, nrows in runs_by_group[g]:
                col0 = side * gn + (start - t0 * P) // P
                ncols = nrows // P
                it = idx_tiles[side]
                nc.gpsimd.dma_gather(
                    out_ap=bass.AP(
                        tensor=M[:].tensor,
                        offset=M[:].offset + col0 * ROWB,
                        ap=[list(M[:].ap[0]), [ROWB, ncols], [1, ROWB]]),
                    in_ap=tbl[bank * BANKR:(bank + 1) * BANKR, :],
                    idxs_ap=it[:, start // 16:(start + nrows) // 16],
                    num_idxs=nrows, num_idxs_reg=nrows, elem_size=ROWB,
                    queue_num=qn[0],
                )
                qn[0] = (qn[0] + 1) % 4
            R = rpool.tile([P, gn * RROW], F16)
            nc.sync.dma_start(R[:], rmat[:, t0 * RROW:(t0 + gn) * RROW])

            Mi = M[:].rearrange("p (b g r) -> p b g r", b=2, g=gn)
            Mf = M[:].bitcast(F16).rearrange("p (b g r) -> p b g r", b=2, g=gn)
            U = upool.tile([P, 2 * gn * 216], F16)
            U4 = U[:].rearrange("p (b g r) -> p b g r", b=2, g=gn)
            PH = ppool.tile([P, 2 * gn * 216], F16)
            PH4 = PH[:].rearrange("p (b g r) -> p b g r", b=2, g=gn)
            nc.scalar.activation(
                out=U4, in_=Mi[:, :, :, OFF_FRQ:OFF_FRQ + 216],
                func=mybir.ActivationFunctionType.Copy)
            nc.scalar.activation(
                out=PH4, in_=Mi[:, :, :, OFF_PHI:OFF_PHI + 216],
                func=mybir.ActivationFunctionType.Copy)
            st[g] = dict(M=M, R=R, Mi=Mi, Mf=Mf, U=U, U4=U4, PH=PH, PH4=PH4,
                         t0=t0, gn=gn)

        def phase2(g):
            """DVE: U' pipeline + e-products (fills the Sin latency)."""
            v = st[g]
            t0, gn, U, U4, PH4 = v["t0"], v["gn"], v["U"], v["U4"], v["PH4"]
            Mf, R = v["Mf"], v["R"]
            for k in range(3):
                Uk = bass.AP(
                    tensor=U[:].tensor, offset=U[:].offset + 72 * k,
                    ap=[list(U[:].ap[0]), [gn * 216, 2], [216, gn], [1, 72]])
                tck = tv_t[:, 3 * t0 + k:3 * t0 + k + 1]
                tvk = bass.AP(
                    tensor=tck.tensor, offset=tck.offset,
                    ap=[list(tck.ap[0]), [0, 2], [3, gn], [0, 72]])
                nc.vector.tensor_tensor(Uk, Uk, tvk, AluOpType.mult)
            nc.vector.tensor_tensor(U4, U4, PH4, AluOpType.add)
            # range-reduce day plane only (|U'*QS| can reach ~6.4 there)
            Ud = bass.AP(
                tensor=U[:].tensor, offset=U[:].offset + 144,
                ap=[list(U[:].ap[0]), [gn * 216, 2], [216, gn], [1, 72]])
            QI = qpool.tile([P, 2 * gn * 72], I16)
            QI4 = QI[:].rearrange("p (b g r) -> p b g r", b=2, g=gn)
            nc.vector.tensor_scalar(
                out=QI4, in0=Ud, scalar1=float(QS / TWO_PI), scalar2=None,
                op0=AluOpType.mult)
            nc.vector.scalar_tensor_tensor(
                out=Ud, in0=QI4, scalar=float(-TWO_PI / QS), in1=Ud,
                op0=AluOpType.mult, op1=AluOpType.add)
            # e-products while ACT runs Sin:
            # op1a: Z[:, :, {0,100}+0:64] = es|eo (s-row, f16) * R
            Z = zpool.tile([P, gn * RROW], F16)
            Zf = Z[:]
            Ze = bass.AP(tensor=Zf.tensor, offset=Zf.offset,
                         ap=[list(Zf.ap[0]), [RROW, gn], [100, 2], [1, 64]])
            Me_s = bass.AP(tensor=Mf.tensor, offset=Mf.offset,
                           ap=[list(Mf.ap[0]), [512, gn], [64, 2], [1, 64]])
            Re = bass.AP(tensor=R[:].tensor, offset=R[:].offset,
                         ap=[list(R[:].ap[0]), [RROW, gn], [100, 2], [1, 64]])
            nc.vector.tensor_tensor(Ze, Me_s, Re, AluOpType.mult)
            # op2 e-part: multiply with swapped o-row e factors
            Z3 = Zf.rearrange("p (g r) -> p g r", g=gn)
            ofs_f = gn * 512
            Meo_o = bass.AP(tensor=Mf.tensor, offset=Mf.offset + ofs_f + 64,
                            ap=[list(Mf.ap[0]), [512, gn], [1, 64]])
            Mes_o = bass.AP(tensor=Mf.tensor, offset=Mf.offset + ofs_f,
                            ap=[list(Mf.ap[0]), [512, gn], [1, 64]])
            nc.vector.tensor_tensor(
                Z3[:, :, 0:64], Z3[:, :, 0:64], Meo_o, AluOpType.mult)
            nc.vector.tensor_tensor(
                Z3[:, :, 100:164], Z3[:, :, 100:164], Mes_o, AluOpType.mult)
            v["Z"] = Z
            v["Z3"] = Z3

        def sin_amp(g):
            """ACT: Sin, then amp cast (scaled by QS) into the dead U."""
            v = st[g]
            gn, U4, Mi = v["gn"], v["U4"], v["Mi"]
            S = spool.tile([P, 2 * gn * 216], F16)
            S4 = S[:].rearrange("p (b g r) -> p b g r", b=2, g=gn)
            nc.scalar.activation(
                out=S4, in_=U4, func=mybir.ActivationFunctionType.Sin,
                scale=float(QS))
            # amp * QS so T is real-valued and op1b needs no extra scale
            nc.scalar.activation(
                out=U4, in_=Mi[:, :, :, OFF_AMP:OFF_AMP + 216],
                func=mybir.ActivationFunctionType.Copy, scale=float(QS))
            v["S"] = S
            v["S4"] = S4

        def phase3(g):
            """DVE: amp mul, T sums, temporal products, fold."""
            v = st.pop(g)
            t0, gn = v["t0"], v["gn"]
            S, S4, U4, R, Z3 = v["S"], v["S4"], v["U4"], v["R"], v["Z3"]
            Z = v["Z"]
            nc.vector.tensor_tensor(S4, S4, U4, AluOpType.mult)
            T = tpool.tile([P, 2 * gn * 72], F16)
            T4 = T[:].rearrange("p (b g r) -> p b g r", b=2, g=gn)
            Sk = [bass.AP(
                tensor=S[:].tensor, offset=S[:].offset + 72 * k,
                ap=[list(S[:].ap[0]), [gn * 216, 2], [216, gn], [1, 72]])
                for k in range(3)]
            nc.vector.tensor_tensor(T4, Sk[0], Sk[1], AluOpType.add)
            nc.vector.tensor_tensor(T4, T4, Sk[2], AluOpType.add)
            # op1b: Z[:, :, {64,164}+0:36] = T_s * R  (T already real)
            Zf = Z[:]
            Zt = bass.AP(tensor=Zf.tensor, offset=Zf.offset + 64,
                         ap=[list(Zf.ap[0]), [RROW, gn], [100, 2], [1, 36]])
            Ts = bass.AP(tensor=T[:].tensor, offset=T[:].offset,
                         ap=[list(T[:].ap[0]), [72, gn], [36, 2], [1, 36]])
            Rt = bass.AP(tensor=R[:].tensor, offset=R[:].offset + 64,
                         ap=[list(R[:].ap[0]), [RROW, gn], [100, 2], [1, 36]])
            nc.vector.tensor_tensor(Zt, Ts, Rt, AluOpType.mult)
            # op2 temporal part with swapped o-row T factors
            To_o = bass.AP(tensor=T[:].tensor,
                           offset=T[:].offset + gn * 72 + 36,
                           ap=[list(T[:].ap[0]), [72, gn], [1, 36]])
            Ts_o = bass.AP(tensor=T[:].tensor, offset=T[:].offset + gn * 72,
                           ap=[list(T[:].ap[0]), [72, gn], [1, 36]])
            nc.vector.tensor_tensor(
                Z3[:, :, 64:100], Z3[:, :, 64:100], To_o, AluOpType.mult)
            nc.vector.tensor_tensor(
                Z3[:, :, 164:200], Z3[:, :, 164:200], Ts_o, AluOpType.mult)
            acc_v = oacc[:, t0:t0 + gn].rearrange("p (g r) -> p g r", r=1)
            nc.vector.tensor_reduce(
                out=acc_v, in_=Z3, axis=mybir.AxisListType.X, op=AluOpType.add)

        phase1(0)
        for g in range(ngroups):
            phase2(g)
            sin_amp(g)
            if g + 1 < ngroups:
                phase1(g + 1)
            phase3(g)

        nc.sync.dma_start(out[:, :], oacc[:])

    _split_multi_waits(nc)
    nc.finalize()
    return nc


def _split_multi_waits(nc, limit=1):
    """walrus rejects instructions with more than one sync-wait command."""
    n = 0
    for bb in nc.main_func.blocks:
        insts = bb.instructions
        i = 0
        while i < len(insts):
            inst = insts[i]
            si = inst.sync_info
            if si is not None and len(si.on_wait) > limit:
                waits = list(si.on_wait)
                for w in waits[:-limit]:
                    nop = mybir.InstNoOp(name=f"{inst.name}-wsplit{n}",
                                         ins=[], outs=[])
                    n += 1
                    nop.engine = inst.engine
                    nop.sync_info = mybir.SyncInfo(on_wait=[w], on_update=[])
                    nc.register_instruction(nop)
                    insts.insert(i, nop)
                    i += 1
                inst.sync_info = mybir.SyncInfo(
                    on_wait=waits[-limit:], on_update=list(si.on_update))
            i += 1
    return nc


# ----------------------------------------------------------------------------
# host-side packing
# ----------------------------------------------------------------------------

def _q8(x):
    return np.clip(np.rint(np.asarray(x, np.float32) / QS), -127, 127
                   ).astype(np.int8)


def pack_tables(e_s, e_o, amp_s, frq_s, phi_s, amp_o, frq_o, phi_o):
    tbl = np.zeros((NE_PAD, ROWB), np.int8)
    ef = np.concatenate([e_s, e_o], axis=1).astype(np.float16)  # [NE, 128]
    tbl[:NE, 0:256] = ef.view(np.int8)
    for off, ts, to in ((OFF_FRQ, frq_s, frq_o), (OFF_PHI, phi_s, phi_o),
                        (OFF_AMP, amp_s, amp_o)):
        for k in range(3):
            base = off + 72 * k
            tbl[:NE, base:base + 36] = _q8(ts[k])
            tbl[:NE, base + 36:base + 72] = _q8(to[k])
    return tbl


def pack_relations(r_f, r_i):
    # T carries both QS factors (amp cast applies QS; the o-side T_raw*QS
    # comes from the same scaled cast), so all columns share the 256 scale
    rp = np.empty((NR, RROW), np.float32)
    rp[:, 0:64] = 256.0 * r_f[:, 0:64]
    rp[:, 64:100] = 256.0 * r_f[:, 64:100]
    rp[:, 100:164] = 256.0 * r_i[:, 0:64]
    rp[:, 164:200] = 256.0 * r_i[:, 64:100]
    return rp.astype(np.float16)


def plan_layout(s, o):
    """Bucket-sort the full batch; deal each bucket evenly to cores.
    Returns (nt, runs, per-core slot->orig index arrays)."""
    s = np.asarray(s)
    o = np.asarray(o)
    sb = s // BANKR
    ob = o // BANKR
    bucket = sb * NBANK + ob
    order = np.argsort(bucket, kind="stable")
    bsizes = np.bincount(bucket, minlength=NBANK * NBANK)

    caps = np.zeros(NBANK * NBANK, np.int64)
    for b in range(NBANK * NBANK):
        percore = -(-bsizes[b] // NCORES)         # ceil
        caps[b] = -(-percore // P) * P if percore else 0
    nt = int(caps.sum()) // P

    # per-core slot -> original element (-1 = pad)
    slot_orig = np.full((NCORES, nt * P), -1, np.int64)
    bstart = np.zeros(NBANK * NBANK + 1, np.int64)
    np.cumsum(bsizes, out=bstart[1:])
    cstart = np.zeros(NBANK * NBANK + 1, np.int64)
    np.cumsum(caps, out=cstart[1:])
    for b in range(NBANK * NBANK):
        members = order[bstart[b]:bstart[b + 1]]
        for c in range(NCORES):
            part = members[c::NCORES]
            slot_orig[c, cstart[b]:cstart[b] + len(part)] = part

    # gather runs (identical across cores): split at group boundaries and
    # (o-side) bucket boundaries / (s-side) s-bank changes.
    runs = []
    gbound = G * P
    for side in range(2):
        cuts = {0, nt * P}
        for b in range(NBANK * NBANK):
            if caps[b] == 0:
                continue
            cuts.add(int(cstart[b]))
            cuts.add(int(cstart[b + 1]))
        # merge s-side cuts within same s_bank: recompute: only keep cut
        # if bank changes there (or group boundary)
        cl = sorted(cuts)
        segs = []
        for a, bnd in zip(cl[:-1], cl[1:]):
            if a == bnd:
                continue
            # bank of this segment
            bidx = np.searchsorted(cstart, a, side="right") - 1
            bank = (bidx // NBANK) if side == 0 else (bidx % NBANK)
            segs.append((a, bnd, int(bank)))
        # merge adjacent same-bank segments (s-side mostly)
        merged = []
        for a, bnd, bank in segs:
            if merged and merged[-1][2] == bank and merged[-1][1] == a:
                merged[-1] = (merged[-1][0], bnd, bank)
            else:
                merged.append((a, bnd, bank))
        # split at group boundaries and the SWDGE ring capacity (the
        # descriptor carveout holds 1024 descriptors per queue; one gather
        # must not exceed 1024 rows)
        for a, bnd, bank in merged:
            x = a
            while x < bnd:
                nxt = min(bnd, (x // gbound + 1) * gbound, x + 1024)
                runs.append((side, bank, int(x), int(nxt - x)))
                x = nxt
    return nt, runs, slot_orig


def _wrap16(stream):
    # idx i -> [i % 16, i // 16], replicated to all 8 GPSIMD core groups
    return np.ascontiguousarray(np.tile(stream.reshape(-1, 16).T, (8, 1)))


def pack_core_inputs(core, nt, slot_orig, s, o, r, y, m, d, rp):
    so = slot_orig[core]
    valid = so >= 0
    idx = np.where(valid, so, 0)

    sv = np.where(valid, np.asarray(s)[idx], 0).astype(np.int64)
    ov = np.where(valid, np.asarray(o)[idx], 0).astype(np.int64)
    s_rel = (sv % BANKR).astype(np.int16)
    o_rel = (ov % BANKR).astype(np.int16)
    # pads: relative id 0 of the bucket's bank (sv=0 -> rel 0, safe)
    rv = np.where(valid, np.asarray(r)[idx], 0)
    yv = np.where(valid, np.asarray(y)[idx], 0).astype(np.float16)
    mv = np.where(valid, np.asarray(m)[idx], 0).astype(np.float16)
    dv = np.where(valid, np.asarray(d)[idx], 0).astype(np.float16)

    rmat = np.ascontiguousarray(
        rp[rv].reshape(nt, P, RROW).transpose(1, 0, 2).reshape(P, nt * RROW))
    tvs = np.stack([yv, mv, dv], axis=-1)  # [nt*P, 3]
    tv = np.ascontiguousarray(
        tvs.reshape(nt, P, 3).transpose(1, 0, 2).reshape(P, nt * 3))
    return {
        "sidx": _wrap16(s_rel), "oidx": _wrap16(o_rel),
        "rmat": rmat, "tv": tv,
    }


_NC_CACHE = {}


def kernel(s, r, o, y, m, d, e_s, e_o, amp_s, frq_s, phi_s,
           amp_o, frq_o, phi_o, r_f, r_i, _trace=False):
    s = np.asarray(s); r = np.asarray(r); o = np.asarray(o)
    tbl = pack_tables(np.asarray(e_s), np.asarray(e_o), np.asarray(amp_s),
                      np.asarray(frq_s), np.asarray(phi_s), np.asarray(amp_o),
                      np.asarray(frq_o), np.asarray(phi_o))
    rp = pack_relations(np.asarray(r_f), np.asarray(r_i))

    nt, runs, slot_orig = plan_layout(s, o)
    key = (nt, tuple(runs))
    if key not in _NC_CACHE:
        _NC_CACHE.clear()
        _NC_CACHE[key] = build_nc(nt, runs)
    nc = _NC_CACHE[key]

    in_maps = []
    for c in range(NCORES):
        im = pack_core_inputs(c, nt, slot_orig, s, o, r,
                              np.asarray(y), np.asarray(m), np.asarray(d), rp)
        im["tbl"] = tbl
        in_maps.append(im)

    res = run_bass_kernel_spmd(nc, in_maps, list(range(NCORES)), trace=_trace)
    full = np.empty(B, np.float32)
    for c in range(NCORES):
        vals = np.asarray(res.results[c]["out"]).T.reshape(-1)  # slot order
        so = slot_orig[c]
        vld = so >= 0
        full[so[vld]] = vals[vld]
    full /= KOUT
    if _trace:
        return full, res
    return full


# revision 23
# speedup vs baseline: 1.1329x; 1.1329x over previous
"""DESimplE scoring kernel v3 for 8 Trainium2 NeuronCores.

v2 (fp16 mega-table + per-128-row indirect DMAs) ran at ~869 us with
GpSimd descriptor generation as the top serialized engine (512 x 1.3 us)
and DVE at ~530 us. v3 restructures around batched SWDGE gathers and an
int8-quantized table:

  * dma_gather (InstDMAGatherAnt): one instruction gathers hundreds of
    rows (994 ns fixed + 0.34 ns/row) instead of one 128-row indirect
    DMA per 1.3 us. Its int16 row index limits the addressable table to
    32K rows, so the 200K-entity table is split into 7 banks and the
    batch is bucket-sorted host-side by (s_bank, o_bank); each bucket
    chunk is gathered from a single bank base.
  * Rows are 1024 B: e_s|e_o kept fp16 (256 B), frq/phi/amp quantized
    int8 (216 B each, scale QS = 0.2/127, clamped at 4 sigma), 120 B pad
    (dma_gather requires elem_size % 256 == 0). Halves gather bytes.
  * DVE ops read int8 operands directly (mixed-dtype tensor_tensor);
    quant scales are folded into the relation matrix, the activation
    scale of Sin, and one scalar_tensor_tensor.
  * U' = frq_q*t + phi_q is computed in "quant units" so the phi add is
    a plain tensor_tensor; Sin applies the QS scale. |frq| <= 0.2
    guarantees |U| < pi for the year/month planes; only the day plane
    needs the round-to-i16 range reduction.
  * Final 200-column reduction is one DVE tensor_reduce into fp32.
  * The batch is dealt bucket-evenly across cores so the compiled
    program (gather run structure) is identical for all 8 cores (SPMD);
    bucket chunks are padded to 128-slot tiles with real index 0 pads
    (~+10-15% slots), dropped on the host.

Score layout per slot column (Z, 200 fp16):
  [0:64)    es(s) * 256*rf64 * eo(o)
  [64:100)  (Ts(s)*QS) * 256*QS*rf36 * To(o)
  [100:164) eo(s) * 256*ri64 * es(o)
  [164:200) (To(s)*QS) * 256*QS*ri36 * Ts(o)
Host divides the accumulated output by 512 (256 = 0.5*512 folded into
the relation columns).
"""

import numpy as np
from contextlib import ExitStack

import concourse.bass as bass
import concourse.bacc as bacc
import concourse.tile as tile
from concourse import mybir
from concourse.alu_op_type import AluOpType
from concourse.bass_utils import run_bass_kernel_spmd

NE, NR, B = 200000, 500, 262144
S_DIM, T_DIM = 64, 36
NCORES = 8
P = 128
BC = B // NCORES            # 32768 elements per core
NBANK = 7
BANKR = 28672               # rows per bank; 7*28672 = 200704 >= NE
NE_PAD = NBANK * BANKR
ROWB = 1024                 # bytes per table row
OFF_FRQ, OFF_PHI, OFF_AMP = 256, 472, 688
QS = 0.2 / 127.0            # int8 quant scale for frq/phi/amp
RROW = 200
KOUT = 512.0
G = 12                      # tiles per pipeline group
TWO_PI = float(2 * np.pi)
TADD_GPS = False            # GPS TT is in a different ucode library than
                            # dma_gather -> per-group library reloads; keep off
FOLD_GPS = False            # GPS tensor_reduce is partition-axis only

F32 = mybir.dt.float32
F16 = mybir.dt.float16
I16 = mybir.dt.int16
I8 = mybir.dt.int8


# ----------------------------------------------------------------------------
# device program
# ----------------------------------------------------------------------------

def build_nc(nt, runs):
    """nt: total tiles per core. runs: list of
    (side, bank, slot_start, nrows) with slot_start/nrows multiples of
    128, pre-split at group boundaries, identical across cores."""
    nc = bacc.Bacc(num_swdge_queues=4)

    tbl = nc.declare_dram_parameter("tbl", [NE_PAD, ROWB], I8, isOutput=False)
    # idx streams are wrapped in 16 partitions and replicated 8x (one copy
    # per GPSIMD core's partition group)
    sidx = nc.declare_dram_parameter("sidx", [P, nt * 8], I16, isOutput=False)
    oidx = nc.declare_dram_parameter("oidx", [P, nt * 8], I16, isOutput=False)
    rmat = nc.declare_dram_parameter("rmat", [P, nt * RROW], F16, isOutput=False)
    tv = nc.declare_dram_parameter("tv", [P, nt * 3], F16, isOutput=False)
    out = nc.declare_dram_parameter("out", [P, nt], F32, isOutput=True)

    ngroups = (nt + G - 1) // G
    # bin runs by group
    runs_by_group = [[] for _ in range(ngroups)]
    for side, bank, start, nrows in runs:
        g = start // (G * P)
        assert (start + nrows - 1) // (G * P) == g
        runs_by_group[g].append((side, bank, start, nrows))

    with ExitStack() as ctx:
        tc = ctx.enter_context(tile.TileContext(nc))
        cpool = ctx.enter_context(tc.tile_pool(name="const", bufs=1))
        mpool = ctx.enter_context(tc.tile_pool(name="m", bufs=3))
        rpool = ctx.enter_context(tc.tile_pool(name="r", bufs=2))
        upool = ctx.enter_context(tc.tile_pool(name="u", bufs=2))
        ppool = ctx.enter_context(tc.tile_pool(name="ph", bufs=2))
        qpool = ctx.enter_context(tc.tile_pool(name="q", bufs=2))
        spool = ctx.enter_context(tc.tile_pool(name="s", bufs=2))
        tpool = ctx.enter_context(tc.tile_pool(name="t", bufs=2))
        zpool = ctx.enter_context(tc.tile_pool(name="z", bufs=2))

        sidx_t = cpool.tile([P, nt * 8], I16)
        nc.sync.dma_start(sidx_t[:], sidx[:, :])
        oidx_t = cpool.tile([P, nt * 8], I16)
        nc.sync.dma_start(oidx_t[:], oidx[:, :])
        tv_t = cpool.tile([P, nt * 3], F16)
        nc.sync.dma_start(tv_t[:], tv[:, :])
        oacc = cpool.tile([P, nt], F32)
        idx_tiles = {0: sidx_t, 1: oidx_t}

        qn = [0]
        st = {}  # per-group live tiles

        def phase1(g):
            """gathers + R load + ACT casts for group g."""
            t0 = g * G
            gn = min(G, nt - t0)
            M = mpool.tile([P, 2 * gn * ROWB], I8)
            for side, bank, start, nrows in runs_by_group[g]:
                col0 = side * gn + (start - t0 * P) // P
                ncols = nrows // P
                it = idx_tiles[side]
                nc.gpsimd.dma_gather(
                    out_ap=bass.AP(
                        tensor=M[:].tensor,
                        offset=M[:].offset + col0 * ROWB,
                        ap=[list(M[:].ap[0]), [ROWB, ncols], [1, ROWB]]),
                    in_ap=tbl[bank * BANKR:(bank + 1) * BANKR, :],
                    idxs_ap=it[:, start // 16:(start + nrows) // 16],
                    num_idxs=nrows, num_idxs_reg=nrows, elem_size=ROWB,
                    queue_num=qn[0],
                )
                qn[0] = (qn[0] + 1) % 4
            R = rpool.tile([P, gn * RROW], F16)
            nc.sync.dma_start(R[:], rmat[:, t0 * RROW:(t0 + gn) * RROW])

            Mi = M[:].rearrange("p (b g r) -> p b g r", b=2, g=gn)
            Mf = M[:].bitcast(F16).rearrange("p (b g r) -> p b g r", b=2, g=gn)
            U = upool.tile([P, 2 * gn * 216], F16)
            U4 = U[:].rearrange("p (b g r) -> p b g r", b=2, g=gn)
            PH = ppool.tile([P, 2 * gn * 216], F16)
            PH4 = PH[:].rearrange("p (b g r) -> p b g r", b=2, g=gn)
            nc.scalar.activation(
                out=U4, in_=Mi[:, :, :, OFF_FRQ:OFF_FRQ + 216],
                func=mybir.ActivationFunctionType.Copy)
            nc.scalar.activation(
                out=PH4, in_=Mi[:, :, :, OFF_PHI:OFF_PHI + 216],
                func=mybir.ActivationFunctionType.Copy)
            st[g] = dict(M=M, R=R, Mi=Mi, Mf=Mf, U=U, U4=U4, PH=PH, PH4=PH4,
                         t0=t0, gn=gn)

        def phase2(g):
            """DVE: U' pipeline + e-products (fills the Sin latency)."""
            v = st[g]
            t0, gn, U, U4, PH4 = v["t0"], v["gn"], v["U"], v["U4"], v["PH4"]
            Mf, R = v["Mf"], v["R"]
            for k in range(3):
                Uk = bass.AP(
                    tensor=U[:].tensor, offset=U[:].offset + 72 * k,
                    ap=[list(U[:].ap[0]), [gn * 216, 2], [216, gn], [1, 72]])
                tck = tv_t[:, 3 * t0 + k:3 * t0 + k + 1]
                tvk = bass.AP(
                    tensor=tck.tensor, offset=tck.offset,
                    ap=[list(tck.ap[0]), [0, 2], [3, gn], [0, 72]])
                nc.vector.tensor_tensor(Uk, Uk, tvk, AluOpType.mult)
            nc.vector.tensor_tensor(U4, U4, PH4, AluOpType.add)
            # range-reduce day plane only (|U'*QS| can reach ~6.4 there)
            Ud = bass.AP(
                tensor=U[:].tensor, offset=U[:].offset + 144,
                ap=[list(U[:].ap[0]), [gn * 216, 2], [216, gn], [1, 72]])
            QI = qpool.tile([P, 2 * gn * 72], I16)
            QI4 = QI[:].rearrange("p (b g r) -> p b g r", b=2, g=gn)
            nc.vector.tensor_scalar(
                out=QI4, in0=Ud, scalar1=float(QS / TWO_PI), scalar2=None,
                op0=AluOpType.mult)
            nc.vector.scalar_tensor_tensor(
                out=Ud, in0=QI4, scalar=float(-TWO_PI / QS), in1=Ud,
                op0=AluOpType.mult, op1=AluOpType.add)
            # e-products while ACT runs Sin:
            # op1a: Z[:, :, {0,100}+0:64] = es|eo (s-row, f16) * R
            Z = zpool.tile([P, gn * RROW], F16)
            Zf = Z[:]
            Ze = bass.AP(tensor=Zf.tensor, offset=Zf.offset,
                         ap=[list(Zf.ap[0]), [RROW, gn], [100, 2], [1, 64]])
            Me_s = bass.AP(tensor=Mf.tensor, offset=Mf.offset,
                           ap=[list(Mf.ap[0]), [512, gn], [64, 2], [1, 64]])
            Re = bass.AP(tensor=R[:].tensor, offset=R[:].offset,
                         ap=[list(R[:].ap[0]), [RROW, gn], [100, 2], [1, 64]])
            nc.vector.tensor_tensor(Ze, Me_s, Re, AluOpType.mult)
            # op2 e-part: multiply with swapped o-row e factors
            Z3 = Zf.rearrange("p (g r) -> p g r", g=gn)
            ofs_f = gn * 512
            Meo_o = bass.AP(tensor=Mf.tensor, offset=Mf.offset + ofs_f + 64,
                            ap=[list(Mf.ap[0]), [512, gn], [1, 64]])
            Mes_o = bass.AP(tensor=Mf.tensor, offset=Mf.offset + ofs_f,
                            ap=[list(Mf.ap[0]), [512, gn], [1, 64]])
            nc.vector.tensor_tensor(
                Z3[:, :, 0:64], Z3[:, :, 0:64], Meo_o, AluOpType.mult)
            nc.vector.tensor_tensor(
                Z3[:, :, 100:164], Z3[:, :, 100:164], Mes_o, AluOpType.mult)
            v["Z"] = Z
            v["Z3"] = Z3

        def sin_amp(g):
            """ACT: Sin, then amp cast (scaled by QS) into the dead U."""
            v = st[g]
            gn, U4, Mi = v["gn"], v["U4"], v["Mi"]
            S = spool.tile([P, 2 * gn * 216], F16)
            S4 = S[:].rearrange("p (b g r) -> p b g r", b=2, g=gn)
            nc.scalar.activation(
                out=S4, in_=U4, func=mybir.ActivationFunctionType.Sin,
                scale=float(QS))
            # amp * QS so T is real-valued and op1b needs no extra scale
            nc.scalar.activation(
                out=U4, in_=Mi[:, :, :, OFF_AMP:OFF_AMP + 216],
                func=mybir.ActivationFunctionType.Copy, scale=float(QS))
            v["S"] = S
            v["S4"] = S4

        def phase3(g):
            """DVE: amp mul, T sums, temporal products, fold."""
            v = st.pop(g)
            t0, gn = v["t0"], v["gn"]
            S, S4, U4, R, Z3 = v["S"], v["S4"], v["U4"], v["R"], v["Z3"]
            Z = v["Z"]
            nc.vector.tensor_tensor(S4, S4, U4, AluOpType.mult)
            T = tpool.tile([P, 2 * gn * 72], F16)
            T4 = T[:].rearrange("p (b g r) -> p b g r", b=2, g=gn)
            Sk = [bass.AP(
                tensor=S[:].tensor, offset=S[:].offset + 72 * k,
                ap=[list(S[:].ap[0]), [gn * 216, 2], [216, gn], [1, 72]])
                for k in range(3)]
            nc.vector.tensor_tensor(T4, Sk[0], Sk[1], AluOpType.add)
            nc.vector.tensor_tensor(T4, T4, Sk[2], AluOpType.add)
            # op1b: Z[:, :, {64,164}+0:36] = T_s * R  (T already real)
            Zf = Z[:]
            Zt = bass.AP(tensor=Zf.tensor, offset=Zf.offset + 64,
                         ap=[list(Zf.ap[0]), [RROW, gn], [100, 2], [1, 36]])
            Ts = bass.AP(tensor=T[:].tensor, offset=T[:].offset,
                         ap=[list(T[:].ap[0]), [72, gn], [36, 2], [1, 36]])
            Rt = bass.AP(tensor=R[:].tensor, offset=R[:].offset + 64,
                         ap=[list(R[:].ap[0]), [RROW, gn], [100, 2], [1, 36]])
            nc.vector.tensor_tensor(Zt, Ts, Rt, AluOpType.mult)
            # op2 temporal part with swapped o-row T factors
            To_o = bass.AP(tensor=T[:].tensor,
                           offset=T[:].offset + gn * 72 + 36,
                           ap=[list(T[:].ap[0]), [72, gn], [1, 36]])
            Ts_o = bass.AP(tensor=T[:].tensor, offset=T[:].offset + gn * 72,
                           ap=[list(T[:].ap[0]), [72, gn], [1, 36]])
            nc.vector.tensor_tensor(
                Z3[:, :, 64:100], Z3[:, :, 64:100], To_o, AluOpType.mult)
            nc.vector.tensor_tensor(
                Z3[:, :, 164:200], Z3[:, :, 164:200], Ts_o, AluOpType.mult)
            acc_v = oacc[:, t0:t0 + gn].rearrange("p (g r) -> p g r", r=1)
            nc.vector.tensor_reduce(
                out=acc_v, in_=Z3, axis=mybir.AxisListType.X, op=AluOpType.add)

        for g in range(ngroups):
            phase1(g)
            phase2(g)
            sin_amp(g)
            phase3(g)

        nc.sync.dma_start(out[:, :], oacc[:])

    _split_multi_waits(nc)
    nc.finalize()
    return nc


def _split_multi_waits(nc, limit=1):
    """walrus rejects instructions with more than one sync-wait command."""
    n = 0
    for bb in nc.main_func.blocks:
        insts = bb.instructions
        i = 0
        while i < len(insts):
            inst = insts[i]
            si = inst.sync_info
            if si is not None and len(si.on_wait) > limit:
                waits = list(si.on_wait)
                for w in waits[:-limit]:
                    nop = mybir.InstNoOp(name=f"{inst.name}-wsplit{n}",
                                         ins=[], outs=[])
                    n += 1
                    nop.engine = inst.engine
                    nop.sync_info = mybir.SyncInfo(on_wait=[w], on_update=[])
                    nc.register_instruction(nop)
                    insts.insert(i, nop)
                    i += 1
                inst.sync_info = mybir.SyncInfo(
                    on_wait=waits[-limit:], on_update=list(si.on_update))
            i += 1
    return nc


# ----------------------------------------------------------------------------
# host-side packing
# ----------------------------------------------------------------------------

def _q8(x):
    return np.clip(np.rint(np.asarray(x, np.float32) / QS), -127, 127
                   ).astype(np.int8)


def pack_tables(e_s, e_o, amp_s, frq_s, phi_s, amp_o, frq_o, phi_o):
    tbl = np.zeros((NE_PAD, ROWB), np.int8)
    ef = np.concatenate([e_s, e_o], axis=1).astype(np.float16)  # [NE, 128]
    tbl[:NE, 0:256] = ef.view(np.int8)
    for off, ts, to in ((OFF_FRQ, frq_s, frq_o), (OFF_PHI, phi_s, phi_o),
                        (OFF_AMP, amp_s, amp_o)):
        for k in range(3):
            base = off + 72 * k
            tbl[:NE, base:base + 36] = _q8(ts[k])
            tbl[:NE, base + 36:base + 72] = _q8(to[k])
    return tbl


def pack_relations(r_f, r_i):
    # T carries both QS factors (amp cast applies QS; the o-side T_raw*QS
    # comes from the same scaled cast), so all columns share the 256 scale
    rp = np.empty((NR, RROW), np.float32)
    rp[:, 0:64] = 256.0 * r_f[:, 0:64]
    rp[:, 64:100] = 256.0 * r_f[:, 64:100]
    rp[:, 100:164] = 256.0 * r_i[:, 0:64]
    rp[:, 164:200] = 256.0 * r_i[:, 64:100]
    return rp.astype(np.float16)


def plan_layout(s, o):
    """Bucket-sort the full batch; deal each bucket evenly to cores.
    Returns (nt, runs, per-core slot->orig index arrays)."""
    s = np.asarray(s)
    o = np.asarray(o)
    sb = s // BANKR
    ob = o // BANKR
    bucket = sb * NBANK + ob
    order = np.argsort(bucket, kind="stable")
    bsizes = np.bincount(bucket, minlength=NBANK * NBANK)

    caps = np.zeros(NBANK * NBANK, np.int64)
    for b in range(NBANK * NBANK):
        percore = -(-bsizes[b] // NCORES)         # ceil
        caps[b] = -(-percore // P) * P if percore else 0
    nt = int(caps.sum()) // P

    # per-core slot -> original element (-1 = pad)
    slot_orig = np.full((NCORES, nt * P), -1, np.int64)
    bstart = np.zeros(NBANK * NBANK + 1, np.int64)
    np.cumsum(bsizes, out=bstart[1:])
    cstart = np.zeros(NBANK * NBANK + 1, np.int64)
    np.cumsum(caps, out=cstart[1:])
    for b in range(NBANK * NBANK):
        members = order[bstart[b]:bstart[b + 1]]
        for c in range(NCORES):
            part = members[c::NCORES]
            slot_orig[c, cstart[b]:cstart[b] + len(part)] = part

    # gather runs (identical across cores): split at group boundaries and
    # (o-side) bucket boundaries / (s-side) s-bank changes.
    runs = []
    gbound = G * P
    for side in range(2):
        cuts = {0, nt * P}
        for b in range(NBANK * NBANK):
            if caps[b] == 0:
                continue
            cuts.add(int(cstart[b]))
            cuts.add(int(cstart[b + 1]))
        # merge s-side cuts within same s_bank: recompute: only keep cut
        # if bank changes there (or group boundary)
        cl = sorted(cuts)
        segs = []
        for a, bnd in zip(cl[:-1], cl[1:]):
            if a == bnd:
                continue
            # bank of this segment
            bidx = np.searchsorted(cstart, a, side="right") - 1
            bank = (bidx // NBANK) if side == 0 else (bidx % NBANK)
            segs.append((a, bnd, int(bank)))
        # merge adjacent same-bank segments (s-side mostly)
        merged = []
        for a, bnd, bank in segs:
            if merged and merged[-1][2] == bank and merged[-1][1] == a:
                merged[-1] = (merged[-1][0], bnd, bank)
            else:
                merged.append((a, bnd, bank))
        # split at group boundaries and the SWDGE ring capacity (the
        # descriptor carveout holds 1024 descriptors per queue; one gather
        # must not exceed 1024 rows)
        for a, bnd, bank in merged:
            x = a
            while x < bnd:
                nxt = min(bnd, (x // gbound + 1) * gbound, x + 1024)
                runs.append((side, bank, int(x), int(nxt - x)))
                x = nxt
    return nt, runs, slot_orig


def _wrap16(stream):
    # idx i -> [i % 16, i // 16], replicated to all 8 GPSIMD core groups
    return np.ascontiguousarray(np.tile(stream.reshape(-1, 16).T, (8, 1)))


def pack_core_inputs(core, nt, slot_orig, s, o, r, y, m, d, rp):
    so = slot_orig[core]
    valid = so >= 0
    idx = np.where(valid, so, 0)

    sv = np.where(valid, np.asarray(s)[idx], 0).astype(np.int64)
    ov = np.where(valid, np.asarray(o)[idx], 0).astype(np.int64)
    s_rel = (sv % BANKR).astype(np.int16)
    o_rel = (ov % BANKR).astype(np.int16)
    # pads: relative id 0 of the bucket's bank (sv=0 -> rel 0, safe)
    rv = np.where(valid, np.asarray(r)[idx], 0)
    yv = np.where(valid, np.asarray(y)[idx], 0).astype(np.float16)
    mv = np.where(valid, np.asarray(m)[idx], 0).astype(np.float16)
    dv = np.where(valid, np.asarray(d)[idx], 0).astype(np.float16)

    rmat = np.ascontiguousarray(
        rp[rv].reshape(nt, P, RROW).transpose(1, 0, 2).reshape(P, nt * RROW))
    tvs = np.stack([yv, mv, dv], axis=-1)  # [nt*P, 3]
    tv = np.ascontiguousarray(
        tvs.reshape(nt, P, 3).transpose(1, 0, 2).reshape(P, nt * 3))
    return {
        "sidx": _wrap16(s_rel), "oidx": _wrap16(o_rel),
        "rmat": rmat, "tv": tv,
    }


_NC_CACHE = {}


def kernel(s, r, o, y, m, d, e_s, e_o, amp_s, frq_s, phi_s,
           amp_o, frq_o, phi_o, r_f, r_i, _trace=False):
    s = np.asarray(s); r = np.asarray(r); o = np.asarray(o)
    tbl = pack_tables(np.asarray(e_s), np.asarray(e_o), np.asarray(amp_s),
                      np.asarray(frq_s), np.asarray(phi_s), np.asarray(amp_o),
                      np.asarray(frq_o), np.asarray(phi_o))
    rp = pack_relations(np.asarray(r_f), np.asarray(r_i))

    nt, runs, slot_orig = plan_layout(s, o)
    key = (nt, tuple(runs))
    if key not in _NC_CACHE:
        _NC_CACHE.clear()
        _NC_CACHE[key] = build_nc(nt, runs)
    nc = _NC_CACHE[key]

    in_maps = []
    for c in range(NCORES):
        im = pack_core_inputs(c, nt, slot_orig, s, o, r,
                              np.asarray(y), np.asarray(m), np.asarray(d), rp)
        im["tbl"] = tbl
        in_maps.append(im)

    res = run_bass_kernel_spmd(nc, in_maps, list(range(NCORES)), trace=_trace)
    full = np.empty(B, np.float32)
    for c in range(NCORES):
        vals = np.asarray(res.results[c]["out"]).T.reshape(-1)  # slot order
        so = slot_orig[c]
        vld = so >= 0
        full[so[vld]] = vals[vld]
    full /= KOUT
    if _trace:
        return full, res
    return full


# revision 24
# speedup vs baseline: 1.1491x; 1.0143x over previous
"""DESimplE scoring kernel v3 for 8 Trainium2 NeuronCores.

v2 (fp16 mega-table + per-128-row indirect DMAs) ran at ~869 us with
GpSimd descriptor generation as the top serialized engine (512 x 1.3 us)
and DVE at ~530 us. v3 restructures around batched SWDGE gathers and an
int8-quantized table:

  * dma_gather (InstDMAGatherAnt): one instruction gathers hundreds of
    rows (994 ns fixed + 0.34 ns/row) instead of one 128-row indirect
    DMA per 1.3 us. Its int16 row index limits the addressable table to
    32K rows, so the 200K-entity table is split into 7 banks and the
    batch is bucket-sorted host-side by (s_bank, o_bank); each bucket
    chunk is gathered from a single bank base.
  * Rows are 1024 B: e_s|e_o kept fp16 (256 B), frq/phi/amp quantized
    int8 (216 B each, scale QS = 0.2/127, clamped at 4 sigma), 120 B pad
    (dma_gather requires elem_size % 256 == 0). Halves gather bytes.
  * DVE ops read int8 operands directly (mixed-dtype tensor_tensor);
    quant scales are folded into the relation matrix, the activation
    scale of Sin, and one scalar_tensor_tensor.
  * U' = frq_q*t + phi_q is computed in "quant units" so the phi add is
    a plain tensor_tensor; Sin applies the QS scale. |frq| <= 0.2
    guarantees |U| < pi for the year/month planes; only the day plane
    needs the round-to-i16 range reduction.
  * Final 200-column reduction is one DVE tensor_reduce into fp32.
  * The batch is dealt bucket-evenly across cores so the compiled
    program (gather run structure) is identical for all 8 cores (SPMD);
    bucket chunks are padded to 128-slot tiles with real index 0 pads
    (~+10-15% slots), dropped on the host.

Score layout per slot column (Z, 200 fp16):
  [0:64)    es(s) * 256*rf64 * eo(o)
  [64:100)  (Ts(s)*QS) * 256*QS*rf36 * To(o)
  [100:164) eo(s) * 256*ri64 * es(o)
  [164:200) (To(s)*QS) * 256*QS*ri36 * Ts(o)
Host divides the accumulated output by 512 (256 = 0.5*512 folded into
the relation columns).
"""

import numpy as np
from contextlib import ExitStack

import concourse.bass as bass
import concourse.bacc as bacc
import concourse.tile as tile
from concourse import mybir
from concourse.alu_op_type import AluOpType
from concourse.bass_utils import run_bass_kernel_spmd

NE, NR, B = 200000, 500, 262144
S_DIM, T_DIM = 64, 36
NCORES = 8
P = 128
BC = B // NCORES            # 32768 elements per core
NBANK = 7
BANKR = 28672               # rows per bank; 7*28672 = 200704 >= NE
NE_PAD = NBANK * BANKR
ROWB = 1024                 # bytes per table row
OFF_FRQ, OFF_PHI, OFF_AMP = 256, 472, 688
QS = 0.2 / 127.0            # int8 quant scale for frq/phi/amp
RROW = 200
KOUT = 512.0
G = 12                      # tiles per pipeline group
TWO_PI = float(2 * np.pi)
TADD_GPS = False            # GPS TT is in a different ucode library than
                            # dma_gather -> per-group library reloads; keep off
FOLD_GPS = False            # GPS tensor_reduce is partition-axis only

F32 = mybir.dt.float32
F16 = mybir.dt.float16
I16 = mybir.dt.int16
I8 = mybir.dt.int8


# ----------------------------------------------------------------------------
# device program
# ----------------------------------------------------------------------------

def build_nc(nt, runs):
    """nt: total tiles per core. runs: list of
    (side, bank, slot_start, nrows) with slot_start/nrows multiples of
    128, pre-split at group boundaries, identical across cores."""
    nc = bacc.Bacc(num_swdge_queues=4)

    tbl = nc.declare_dram_parameter("tbl", [NE_PAD, ROWB], I8, isOutput=False)
    # idx streams are wrapped in 16 partitions and replicated 8x (one copy
    # per GPSIMD core's partition group)
    sidx = nc.declare_dram_parameter("sidx", [P, nt * 8], I16, isOutput=False)
    oidx = nc.declare_dram_parameter("oidx", [P, nt * 8], I16, isOutput=False)
    rmat = nc.declare_dram_parameter("rmat", [P, nt * RROW], F16, isOutput=False)
    tv = nc.declare_dram_parameter("tv", [P, nt * 3], F16, isOutput=False)
    out = nc.declare_dram_parameter("out", [P, nt], F32, isOutput=True)

    ngroups = (nt + G - 1) // G
    # bin runs by group
    runs_by_group = [[] for _ in range(ngroups)]
    for side, bank, start, nrows in runs:
        g = start // (G * P)
        assert (start + nrows - 1) // (G * P) == g
        runs_by_group[g].append((side, bank, start, nrows))

    with ExitStack() as ctx:
        tc = ctx.enter_context(tile.TileContext(nc))
        cpool = ctx.enter_context(tc.tile_pool(name="const", bufs=1))
        mpool = ctx.enter_context(tc.tile_pool(name="m", bufs=3))
        rpool = ctx.enter_context(tc.tile_pool(name="r", bufs=2))
        upool = ctx.enter_context(tc.tile_pool(name="u", bufs=2))
        ppool = ctx.enter_context(tc.tile_pool(name="ph", bufs=2))
        qpool = ctx.enter_context(tc.tile_pool(name="q", bufs=2))
        spool = ctx.enter_context(tc.tile_pool(name="s", bufs=2))
        tpool = ctx.enter_context(tc.tile_pool(name="t", bufs=2))
        zpool = ctx.enter_context(tc.tile_pool(name="z", bufs=2))

        sidx_t = cpool.tile([P, nt * 8], I16)
        nc.sync.dma_start(sidx_t[:], sidx[:, :])
        oidx_t = cpool.tile([P, nt * 8], I16)
        nc.sync.dma_start(oidx_t[:], oidx[:, :])
        tv_t = cpool.tile([P, nt * 3], F16)
        nc.sync.dma_start(tv_t[:], tv[:, :])
        oacc = cpool.tile([P, nt], F32)
        idx_tiles = {0: sidx_t, 1: oidx_t}

        qn = [0]
        st = {}  # per-group live tiles

        def phase1(g):
            """gathers + R load + ACT casts for group g."""
            t0 = g * G
            gn = min(G, nt - t0)
            M = mpool.tile([P, 2 * gn * ROWB], I8)
            for side, bank, start, nrows in runs_by_group[g]:
                col0 = side * gn + (start - t0 * P) // P
                ncols = nrows // P
                it = idx_tiles[side]
                nc.gpsimd.dma_gather(
                    out_ap=bass.AP(
                        tensor=M[:].tensor,
                        offset=M[:].offset + col0 * ROWB,
                        ap=[list(M[:].ap[0]), [ROWB, ncols], [1, ROWB]]),
                    in_ap=tbl[bank * BANKR:(bank + 1) * BANKR, :],
                    idxs_ap=it[:, start // 16:(start + nrows) // 16],
                    num_idxs=nrows, num_idxs_reg=nrows, elem_size=ROWB,
                    queue_num=qn[0],
                )
                qn[0] = (qn[0] + 1) % 4
            R = rpool.tile([P, gn * RROW], F16)
            nc.sync.dma_start(R[:], rmat[:, t0 * RROW:(t0 + gn) * RROW])

            Mi = M[:].rearrange("p (b g r) -> p b g r", b=2, g=gn)
            Mf = M[:].bitcast(F16).rearrange("p (b g r) -> p b g r", b=2, g=gn)
            U = upool.tile([P, 2 * gn * 216], F16)
            U4 = U[:].rearrange("p (b g r) -> p b g r", b=2, g=gn)
            PH = ppool.tile([P, 2 * gn * 216], F16)
            PH4 = PH[:].rearrange("p (b g r) -> p b g r", b=2, g=gn)
            nc.scalar.activation(
                out=U4, in_=Mi[:, :, :, OFF_FRQ:OFF_FRQ + 216],
                func=mybir.ActivationFunctionType.Copy)
            nc.scalar.activation(
                out=PH4, in_=Mi[:, :, :, OFF_PHI:OFF_PHI + 216],
                func=mybir.ActivationFunctionType.Copy)
            st[g] = dict(M=M, R=R, Mi=Mi, Mf=Mf, U=U, U4=U4, PH=PH, PH4=PH4,
                         t0=t0, gn=gn)

        def phase2(g):
            """DVE: U' pipeline + e-products (fills the Sin latency)."""
            v = st[g]
            t0, gn, U, U4, PH4 = v["t0"], v["gn"], v["U"], v["U4"], v["PH4"]
            Mf, R = v["Mf"], v["R"]
            for k in range(3):
                Uk = bass.AP(
                    tensor=U[:].tensor, offset=U[:].offset + 72 * k,
                    ap=[list(U[:].ap[0]), [gn * 216, 2], [216, gn], [1, 72]])
                tck = tv_t[:, 3 * t0 + k:3 * t0 + k + 1]
                tvk = bass.AP(
                    tensor=tck.tensor, offset=tck.offset,
                    ap=[list(tck.ap[0]), [0, 2], [3, gn], [0, 72]])
                nc.vector.tensor_tensor(Uk, Uk, tvk, AluOpType.mult)
            nc.vector.tensor_tensor(U4, U4, PH4, AluOpType.add)
            # range-reduce day plane only (|U'*QS| can reach ~6.4 there)
            Ud = bass.AP(
                tensor=U[:].tensor, offset=U[:].offset + 144,
                ap=[list(U[:].ap[0]), [gn * 216, 2], [216, gn], [1, 72]])
            QI = qpool.tile([P, 2 * gn * 72], I16)
            QI4 = QI[:].rearrange("p (b g r) -> p b g r", b=2, g=gn)
            nc.vector.tensor_scalar(
                out=QI4, in0=Ud, scalar1=float(QS / TWO_PI), scalar2=None,
                op0=AluOpType.mult)
            nc.vector.scalar_tensor_tensor(
                out=Ud, in0=QI4, scalar=float(-TWO_PI / QS), in1=Ud,
                op0=AluOpType.mult, op1=AluOpType.add)
            # e-products while ACT runs Sin:
            # op1a: Z[:, :, {0,100}+0:64] = es|eo (s-row, f16) * R
            Z = zpool.tile([P, gn * RROW], F16)
            Zf = Z[:]
            Ze = bass.AP(tensor=Zf.tensor, offset=Zf.offset,
                         ap=[list(Zf.ap[0]), [RROW, gn], [100, 2], [1, 64]])
            Me_s = bass.AP(tensor=Mf.tensor, offset=Mf.offset,
                           ap=[list(Mf.ap[0]), [512, gn], [64, 2], [1, 64]])
            Re = bass.AP(tensor=R[:].tensor, offset=R[:].offset,
                         ap=[list(R[:].ap[0]), [RROW, gn], [100, 2], [1, 64]])
            nc.vector.tensor_tensor(Ze, Me_s, Re, AluOpType.mult)
            # op2 e-part: multiply with swapped o-row e factors
            Z3 = Zf.rearrange("p (g r) -> p g r", g=gn)
            ofs_f = gn * 512
            Meo_o = bass.AP(tensor=Mf.tensor, offset=Mf.offset + ofs_f + 64,
                            ap=[list(Mf.ap[0]), [512, gn], [1, 64]])
            Mes_o = bass.AP(tensor=Mf.tensor, offset=Mf.offset + ofs_f,
                            ap=[list(Mf.ap[0]), [512, gn], [1, 64]])
            nc.vector.tensor_tensor(
                Z3[:, :, 0:64], Z3[:, :, 0:64], Meo_o, AluOpType.mult)
            nc.vector.tensor_tensor(
                Z3[:, :, 100:164], Z3[:, :, 100:164], Mes_o, AluOpType.mult)
            v["Z"] = Z
            v["Z3"] = Z3

        def sin_amp(g):
            """ACT: Sin, then amp cast (scaled by QS) into the dead U."""
            v = st[g]
            gn, U4, Mi = v["gn"], v["U4"], v["Mi"]
            S = spool.tile([P, 2 * gn * 216], F16)
            S4 = S[:].rearrange("p (b g r) -> p b g r", b=2, g=gn)
            nc.scalar.activation(
                out=S4, in_=U4, func=mybir.ActivationFunctionType.Sin,
                scale=float(QS))
            nc.scalar.activation(
                out=U4, in_=Mi[:, :, :, OFF_AMP:OFF_AMP + 216],
                func=mybir.ActivationFunctionType.Copy)
            v["S"] = S
            v["S4"] = S4

        def phase3(g):
            """DVE: amp mul, T sums, temporal products, fold."""
            v = st.pop(g)
            t0, gn = v["t0"], v["gn"]
            S, S4, U4, R, Z3 = v["S"], v["S4"], v["U4"], v["R"], v["Z3"]
            Z = v["Z"]
            nc.vector.tensor_tensor(S4, S4, U4, AluOpType.mult)
            T = tpool.tile([P, 2 * gn * 72], F16)
            T4 = T[:].rearrange("p (b g r) -> p b g r", b=2, g=gn)
            Sk = [bass.AP(
                tensor=S[:].tensor, offset=S[:].offset + 72 * k,
                ap=[list(S[:].ap[0]), [gn * 216, 2], [216, gn], [1, 72]])
                for k in range(3)]
            nc.vector.tensor_tensor(T4, Sk[0], Sk[1], AluOpType.add)
            nc.vector.tensor_tensor(T4, T4, Sk[2], AluOpType.add)
            # op1b: Z[:, :, {64,164}+0:36] = T_s * R  (T already real)
            Zf = Z[:]
            Zt = bass.AP(tensor=Zf.tensor, offset=Zf.offset + 64,
                         ap=[list(Zf.ap[0]), [RROW, gn], [100, 2], [1, 36]])
            Ts = bass.AP(tensor=T[:].tensor, offset=T[:].offset,
                         ap=[list(T[:].ap[0]), [72, gn], [36, 2], [1, 36]])
            Rt = bass.AP(tensor=R[:].tensor, offset=R[:].offset + 64,
                         ap=[list(R[:].ap[0]), [RROW, gn], [100, 2], [1, 36]])
            nc.vector.scalar_tensor_tensor(
                out=Zt, in0=Ts, scalar=float(QS), in1=Rt,
                op0=AluOpType.mult, op1=AluOpType.mult)
            # op2 temporal part with swapped o-row T factors
            To_o = bass.AP(tensor=T[:].tensor,
                           offset=T[:].offset + gn * 72 + 36,
                           ap=[list(T[:].ap[0]), [72, gn], [1, 36]])
            Ts_o = bass.AP(tensor=T[:].tensor, offset=T[:].offset + gn * 72,
                           ap=[list(T[:].ap[0]), [72, gn], [1, 36]])
            nc.vector.tensor_tensor(
                Z3[:, :, 64:100], Z3[:, :, 64:100], To_o, AluOpType.mult)
            nc.vector.tensor_tensor(
                Z3[:, :, 164:200], Z3[:, :, 164:200], Ts_o, AluOpType.mult)
            acc_v = oacc[:, t0:t0 + gn].rearrange("p (g r) -> p g r", r=1)
            nc.vector.tensor_reduce(
                out=acc_v, in_=Z3, axis=mybir.AxisListType.X, op=AluOpType.add)

        for g in range(ngroups):
            phase1(g)
            phase2(g)
            sin_amp(g)
            phase3(g)

        nc.sync.dma_start(out[:, :], oacc[:])

    _split_multi_waits(nc)
    nc.finalize()
    return nc


def _split_multi_waits(nc, limit=1):
    """walrus rejects instructions with more than one sync-wait command."""
    n = 0
    for bb in nc.main_func.blocks:
        insts = bb.instructions
        i = 0
        while i < len(insts):
            inst = insts[i]
            si = inst.sync_info
            if si is not None and len(si.on_wait) > limit:
                waits = list(si.on_wait)
                for w in waits[:-limit]:
                    nop = mybir.InstNoOp(name=f"{inst.name}-wsplit{n}",
                                         ins=[], outs=[])
                    n += 1
                    nop.engine = inst.engine
                    nop.sync_info = mybir.SyncInfo(on_wait=[w], on_update=[])
                    nc.register_instruction(nop)
                    insts.insert(i, nop)
                    i += 1
                inst.sync_info = mybir.SyncInfo(
                    on_wait=waits[-limit:], on_update=list(si.on_update))
            i += 1
    return nc


# ----------------------------------------------------------------------------
# host-side packing
# ----------------------------------------------------------------------------

def _q8(x):
    return np.clip(np.rint(np.asarray(x, np.float32) / QS), -127, 127
                   ).astype(np.int8)


def pack_tables(e_s, e_o, amp_s, frq_s, phi_s, amp_o, frq_o, phi_o):
    tbl = np.zeros((NE_PAD, ROWB), np.int8)
    ef = np.concatenate([e_s, e_o], axis=1).astype(np.float16)  # [NE, 128]
    tbl[:NE, 0:256] = ef.view(np.int8)
    for off, ts, to in ((OFF_FRQ, frq_s, frq_o), (OFF_PHI, phi_s, phi_o),
                        (OFF_AMP, amp_s, amp_o)):
        for k in range(3):
            base = off + 72 * k
            tbl[:NE, base:base + 36] = _q8(ts[k])
            tbl[:NE, base + 36:base + 72] = _q8(to[k])
    return tbl


def pack_relations(r_f, r_i):
    rp = np.empty((NR, RROW), np.float32)
    rp[:, 0:64] = 256.0 * r_f[:, 0:64]
    rp[:, 64:100] = (256.0 * QS) * r_f[:, 64:100]
    rp[:, 100:164] = 256.0 * r_i[:, 0:64]
    rp[:, 164:200] = (256.0 * QS) * r_i[:, 64:100]
    return rp.astype(np.float16)


def plan_layout(s, o):
    """Bucket-sort the full batch; deal each bucket evenly to cores.
    Returns (nt, runs, per-core slot->orig index arrays)."""
    s = np.asarray(s)
    o = np.asarray(o)
    sb = s // BANKR
    ob = o // BANKR
    bucket = sb * NBANK + ob
    order = np.argsort(bucket, kind="stable")
    bsizes = np.bincount(bucket, minlength=NBANK * NBANK)

    caps = np.zeros(NBANK * NBANK, np.int64)
    for b in range(NBANK * NBANK):
        percore = -(-bsizes[b] // NCORES)         # ceil
        caps[b] = -(-percore // P) * P if percore else 0
    nt = int(caps.sum()) // P

    # per-core slot -> original element (-1 = pad)
    slot_orig = np.full((NCORES, nt * P), -1, np.int64)
    bstart = np.zeros(NBANK * NBANK + 1, np.int64)
    np.cumsum(bsizes, out=bstart[1:])
    cstart = np.zeros(NBANK * NBANK + 1, np.int64)
    np.cumsum(caps, out=cstart[1:])
    for b in range(NBANK * NBANK):
        members = order[bstart[b]:bstart[b + 1]]
        for c in range(NCORES):
            part = members[c::NCORES]
            slot_orig[c, cstart[b]:cstart[b] + len(part)] = part

    # gather runs (identical across cores): split at group boundaries and
    # (o-side) bucket boundaries / (s-side) s-bank changes.
    runs = []
    gbound = G * P
    for side in range(2):
        cuts = {0, nt * P}
        for b in range(NBANK * NBANK):
            if caps[b] == 0:
                continue
            cuts.add(int(cstart[b]))
            cuts.add(int(cstart[b + 1]))
        # merge s-side cuts within same s_bank: recompute: only keep cut
        # if bank changes there (or group boundary)
        cl = sorted(cuts)
        segs = []
        for a, bnd in zip(cl[:-1], cl[1:]):
            if a == bnd:
                continue
            # bank of this segment
            bidx = np.searchsorted(cstart, a, side="right") - 1
            bank = (bidx // NBANK) if side == 0 else (bidx % NBANK)
            segs.append((a, bnd, int(bank)))
        # merge adjacent same-bank segments (s-side mostly)
        merged = []
        for a, bnd, bank in segs:
            if merged and merged[-1][2] == bank and merged[-1][1] == a:
                merged[-1] = (merged[-1][0], bnd, bank)
            else:
                merged.append((a, bnd, bank))
        # split at group boundaries and the SWDGE ring capacity (the
        # descriptor carveout holds 1024 descriptors per queue; one gather
        # must not exceed 1024 rows)
        for a, bnd, bank in merged:
            x = a
            while x < bnd:
                nxt = min(bnd, (x // gbound + 1) * gbound, x + 1024)
                runs.append((side, bank, int(x), int(nxt - x)))
                x = nxt
    return nt, runs, slot_orig


def _wrap16(stream):
    # idx i -> [i % 16, i // 16], replicated to all 8 GPSIMD core groups
    return np.ascontiguousarray(np.tile(stream.reshape(-1, 16).T, (8, 1)))


def pack_core_inputs(core, nt, slot_orig, s, o, r, y, m, d, rp):
    so = slot_orig[core]
    valid = so >= 0
    idx = np.where(valid, so, 0)

    sv = np.where(valid, np.asarray(s)[idx], 0).astype(np.int64)
    ov = np.where(valid, np.asarray(o)[idx], 0).astype(np.int64)
    s_rel = (sv % BANKR).astype(np.int16)
    o_rel = (ov % BANKR).astype(np.int16)
    # pads: relative id 0 of the bucket's bank (sv=0 -> rel 0, safe)
    rv = np.where(valid, np.asarray(r)[idx], 0)
    yv = np.where(valid, np.asarray(y)[idx], 0).astype(np.float16)
    mv = np.where(valid, np.asarray(m)[idx], 0).astype(np.float16)
    dv = np.where(valid, np.asarray(d)[idx], 0).astype(np.float16)

    rmat = np.ascontiguousarray(
        rp[rv].reshape(nt, P, RROW).transpose(1, 0, 2).reshape(P, nt * RROW))
    tvs = np.stack([yv, mv, dv], axis=-1)  # [nt*P, 3]
    tv = np.ascontiguousarray(
        tvs.reshape(nt, P, 3).transpose(1, 0, 2).reshape(P, nt * 3))
    return {
        "sidx": _wrap16(s_rel), "oidx": _wrap16(o_rel),
        "rmat": rmat, "tv": tv,
    }


_NC_CACHE = {}


def kernel(s, r, o, y, m, d, e_s, e_o, amp_s, frq_s, phi_s,
           amp_o, frq_o, phi_o, r_f, r_i, _trace=False):
    s = np.asarray(s); r = np.asarray(r); o = np.asarray(o)
    tbl = pack_tables(np.asarray(e_s), np.asarray(e_o), np.asarray(amp_s),
                      np.asarray(frq_s), np.asarray(phi_s), np.asarray(amp_o),
                      np.asarray(frq_o), np.asarray(phi_o))
    rp = pack_relations(np.asarray(r_f), np.asarray(r_i))

    nt, runs, slot_orig = plan_layout(s, o)
    key = (nt, tuple(runs))
    if key not in _NC_CACHE:
        _NC_CACHE.clear()
        _NC_CACHE[key] = build_nc(nt, runs)
    nc = _NC_CACHE[key]

    in_maps = []
    for c in range(NCORES):
        im = pack_core_inputs(c, nt, slot_orig, s, o, r,
                              np.asarray(y), np.asarray(m), np.asarray(d), rp)
        im["tbl"] = tbl
        in_maps.append(im)

    res = run_bass_kernel_spmd(nc, in_maps, list(range(NCORES)), trace=_trace)
    full = np.empty(B, np.float32)
    for c in range(NCORES):
        vals = np.asarray(res.results[c]["out"]).T.reshape(-1)  # slot order
        so = slot_orig[c]
        vld = so >= 0
        full[so[vld]] = vals[vld]
    full /= KOUT
    if _trace:
        return full, res
    return full


# revision 25
# speedup vs baseline: 1.1671x; 1.0156x over previous
"""DESimplE scoring kernel (v6) for 8 Trainium2 NeuronCores.

History: v2 (fp16 mega-table + per-128-row indirect DMAs) ran ~869 us,
serialized on GpSimd SWDGE descriptor generation and DVE. This version
runs ~690 us. Design:

  * dma_gather (InstDMAGatherAnt) batches hundreds of rows per GPSIMD
    instruction. Per-row cost is ~4-8 ns (DMA-engine descriptor
    processing); deep M-tile buffering (bufs=3) keeps transfers
    pipelined (~250 GB/s effective). One gather must stay <= 1024 rows
    (SWDGE descriptor-ring carveout) and its int16 row index limits the
    addressable table to 32K rows, so the 200K-entity table is split
    into 7 banks and the batch is bucket-sorted host-side by
    (s_bank, o_bank); each bucket chunk gathers from one bank base.
    Index streams are 16-partition-wrapped and replicated to all 8
    GPSIMD core groups.
  * Rows are 1024 B: e_s|e_o kept fp16 (256 B), frq/phi/amp quantized
    int8 (216 B each, scale QS = 0.2/127, clamped at 4 sigma), 120 B
    pad (elem_size must be a multiple of 256 B). Halves gather bytes
    vs fp16; measured end-to-end l2 error ~7.4e-3.
  * Engine budget: DVE tensor_tensor runs 2x only when every operand is
    16-bit, so the int8 fields are cast i8->f16 on ACT (0.9 ns/elem
    there vs a 2.3 ns/elem DVE cast or a 1.04 ns/elem 1x mixed mul);
    Sin also runs on ACT with the QS arg scale folded in. GPSIMD
    compute offload is a trap: its tensor ops live in a different
    ucode library than dma_gather, and alternating triggers per-group
    library reloads.
  * U' = frq_q*t + phi_q is computed in quant units (t read via
    stride-0 broadcast APs, no materialized TE tile). |frq| <= 0.2
    bounds |U| < pi for the year/month planes; only the day plane needs
    the round-to-i16 range reduction (RTN i16 cast on DVE, verified).
  * Products write a 200-column Z tile; a single DVE tensor_reduce(X)
    folds it into fp32 per element. The e-part products are emitted
    before Sin so DVE has work during the ACT dependency.
  * The batch is dealt bucket-evenly across cores so the compiled
    program (gather run structure) is identical on all 8 cores (SPMD);
    bucket chunks pad to 128-slot tiles with real index-0 rows
    (~+15% slots), dropped on the host. The program structure depends
    only on bucket sizes, so rebuilding for the same inputs hits the
    on-disk NEFF cache.

Score layout per slot column (Z, 200 fp16):
  [0:64)    es(s) * 256*rf64 * eo(o)
  [64:100)  (Ts(s)*QS) * 256*QS*rf36 * To(o)
  [100:164) eo(s) * 256*ri64 * es(o)
  [164:200) (To(s)*QS) * 256*QS*ri36 * Ts(o)
Host divides the accumulated output by 512 (256 = 0.5*512 folded into
the relation columns).
"""

import numpy as np
from contextlib import ExitStack

import concourse.bass as bass
import concourse.bacc as bacc
import concourse.tile as tile
from concourse import mybir
from concourse.alu_op_type import AluOpType
from concourse.bass_utils import run_bass_kernel_spmd

NE, NR, B = 200000, 500, 262144
S_DIM, T_DIM = 64, 36
NCORES = 8
P = 128
BC = B // NCORES            # 32768 elements per core
NBANK = 7
BANKR = 28672               # rows per bank; 7*28672 = 200704 >= NE
NE_PAD = NBANK * BANKR
ROWB = 1024                 # bytes per table row
OFF_FRQ, OFF_PHI, OFF_AMP = 256, 472, 688
QS = 0.2 / 127.0            # int8 quant scale for frq/phi/amp
RROW = 200
KOUT = 512.0
G = 12                      # tiles per pipeline group
TWO_PI = float(2 * np.pi)
TADD_GPS = False            # GPS TT is in a different ucode library than
                            # dma_gather -> per-group library reloads; keep off
FOLD_GPS = False            # GPS tensor_reduce is partition-axis only

F32 = mybir.dt.float32
F16 = mybir.dt.float16
I16 = mybir.dt.int16
I8 = mybir.dt.int8


# ----------------------------------------------------------------------------
# device program
# ----------------------------------------------------------------------------

def build_nc(nt, runs):
    """nt: total tiles per core. runs: list of
    (side, bank, slot_start, nrows) with slot_start/nrows multiples of
    128, pre-split at group boundaries, identical across cores."""
    nc = bacc.Bacc(num_swdge_queues=4)

    tbl = nc.declare_dram_parameter("tbl", [NE_PAD, ROWB], I8, isOutput=False)
    # idx streams are wrapped in 16 partitions and replicated 8x (one copy
    # per GPSIMD core's partition group)
    sidx = nc.declare_dram_parameter("sidx", [P, nt * 8], I16, isOutput=False)
    oidx = nc.declare_dram_parameter("oidx", [P, nt * 8], I16, isOutput=False)
    rmat = nc.declare_dram_parameter("rmat", [P, nt * RROW], F16, isOutput=False)
    tv = nc.declare_dram_parameter("tv", [P, nt * 3], F16, isOutput=False)
    out = nc.declare_dram_parameter("out", [P, nt], F32, isOutput=True)

    ngroups = (nt + G - 1) // G
    # bin runs by group
    runs_by_group = [[] for _ in range(ngroups)]
    for side, bank, start, nrows in runs:
        g = start // (G * P)
        assert (start + nrows - 1) // (G * P) == g
        runs_by_group[g].append((side, bank, start, nrows))

    with ExitStack() as ctx:
        tc = ctx.enter_context(tile.TileContext(nc))
        cpool = ctx.enter_context(tc.tile_pool(name="const", bufs=1))
        mpool = ctx.enter_context(tc.tile_pool(name="m", bufs=3))
        rpool = ctx.enter_context(tc.tile_pool(name="r", bufs=2))
        upool = ctx.enter_context(tc.tile_pool(name="u", bufs=2))
        ppool = ctx.enter_context(tc.tile_pool(name="ph", bufs=2))
        qpool = ctx.enter_context(tc.tile_pool(name="q", bufs=2))
        spool = ctx.enter_context(tc.tile_pool(name="s", bufs=2))
        tpool = ctx.enter_context(tc.tile_pool(name="t", bufs=2))
        zpool = ctx.enter_context(tc.tile_pool(name="z", bufs=2))

        sidx_t = cpool.tile([P, nt * 8], I16)
        nc.sync.dma_start(sidx_t[:], sidx[:, :])
        oidx_t = cpool.tile([P, nt * 8], I16)
        nc.sync.dma_start(oidx_t[:], oidx[:, :])
        tv_t = cpool.tile([P, nt * 3], F16)
        nc.sync.dma_start(tv_t[:], tv[:, :])
        oacc = cpool.tile([P, nt], F32)
        idx_tiles = {0: sidx_t, 1: oidx_t}

        qn = [0]
        st = {}  # per-group live tiles

        def phase1(g):
            """gathers + R load + ACT casts for group g."""
            t0 = g * G
            gn = min(G, nt - t0)
            M = mpool.tile([P, 2 * gn * ROWB], I8)
            for side, bank, start, nrows in runs_by_group[g]:
                col0 = side * gn + (start - t0 * P) // P
                ncols = nrows // P
                it = idx_tiles[side]
                nc.gpsimd.dma_gather(
                    out_ap=bass.AP(
                        tensor=M[:].tensor,
                        offset=M[:].offset + col0 * ROWB,
                        ap=[list(M[:].ap[0]), [ROWB, ncols], [1, ROWB]]),
                    in_ap=tbl[bank * BANKR:(bank + 1) * BANKR, :],
                    idxs_ap=it[:, start // 16:(start + nrows) // 16],
                    num_idxs=nrows, num_idxs_reg=nrows, elem_size=ROWB,
                    queue_num=qn[0],
                )
                qn[0] = (qn[0] + 1) % 4
            R = rpool.tile([P, gn * RROW], F16)
            nc.sync.dma_start(R[:], rmat[:, t0 * RROW:(t0 + gn) * RROW])

            Mi = M[:].rearrange("p (b g r) -> p b g r", b=2, g=gn)
            Mf = M[:].bitcast(F16).rearrange("p (b g r) -> p b g r", b=2, g=gn)
            U = upool.tile([P, 2 * gn * 216], F16)
            U4 = U[:].rearrange("p (b g r) -> p b g r", b=2, g=gn)
            PH = ppool.tile([P, 2 * gn * 216], F16)
            PH4 = PH[:].rearrange("p (b g r) -> p b g r", b=2, g=gn)
            nc.scalar.activation(
                out=U4, in_=Mi[:, :, :, OFF_FRQ:OFF_FRQ + 216],
                func=mybir.ActivationFunctionType.Copy)
            nc.scalar.activation(
                out=PH4, in_=Mi[:, :, :, OFF_PHI:OFF_PHI + 216],
                func=mybir.ActivationFunctionType.Copy)
            st[g] = dict(M=M, R=R, Mi=Mi, Mf=Mf, U=U, U4=U4, PH=PH, PH4=PH4,
                         t0=t0, gn=gn)

        def phase2(g):
            """DVE: U' pipeline + e-products (fills the Sin latency)."""
            v = st[g]
            t0, gn, U, U4, PH4 = v["t0"], v["gn"], v["U"], v["U4"], v["PH4"]
            Mf, R = v["Mf"], v["R"]
            for k in range(3):
                Uk = bass.AP(
                    tensor=U[:].tensor, offset=U[:].offset + 72 * k,
                    ap=[list(U[:].ap[0]), [gn * 216, 2], [216, gn], [1, 72]])
                tck = tv_t[:, 3 * t0 + k:3 * t0 + k + 1]
                tvk = bass.AP(
                    tensor=tck.tensor, offset=tck.offset,
                    ap=[list(tck.ap[0]), [0, 2], [3, gn], [0, 72]])
                nc.vector.tensor_tensor(Uk, Uk, tvk, AluOpType.mult)
            nc.vector.tensor_tensor(U4, U4, PH4, AluOpType.add)
            # range-reduce day plane only (|U'*QS| can reach ~6.4 there)
            Ud = bass.AP(
                tensor=U[:].tensor, offset=U[:].offset + 144,
                ap=[list(U[:].ap[0]), [gn * 216, 2], [216, gn], [1, 72]])
            QI = qpool.tile([P, 2 * gn * 72], I16)
            QI4 = QI[:].rearrange("p (b g r) -> p b g r", b=2, g=gn)
            nc.vector.tensor_scalar(
                out=QI4, in0=Ud, scalar1=float(QS / TWO_PI), scalar2=None,
                op0=AluOpType.mult)
            nc.vector.scalar_tensor_tensor(
                out=Ud, in0=QI4, scalar=float(-TWO_PI / QS), in1=Ud,
                op0=AluOpType.mult, op1=AluOpType.add)
            # e-products while ACT runs Sin:
            # op1a: Z[:, :, {0,100}+0:64] = es|eo (s-row, f16) * R
            Z = zpool.tile([P, gn * RROW], F16)
            Zf = Z[:]
            Ze = bass.AP(tensor=Zf.tensor, offset=Zf.offset,
                         ap=[list(Zf.ap[0]), [RROW, gn], [100, 2], [1, 64]])
            Me_s = bass.AP(tensor=Mf.tensor, offset=Mf.offset,
                           ap=[list(Mf.ap[0]), [512, gn], [64, 2], [1, 64]])
            Re = bass.AP(tensor=R[:].tensor, offset=R[:].offset,
                         ap=[list(R[:].ap[0]), [RROW, gn], [100, 2], [1, 64]])
            nc.vector.tensor_tensor(Ze, Me_s, Re, AluOpType.mult)
            # op2 e-part: multiply with swapped o-row e factors
            Z3 = Zf.rearrange("p (g r) -> p g r", g=gn)
            ofs_f = gn * 512
            Meo_o = bass.AP(tensor=Mf.tensor, offset=Mf.offset + ofs_f + 64,
                            ap=[list(Mf.ap[0]), [512, gn], [1, 64]])
            Mes_o = bass.AP(tensor=Mf.tensor, offset=Mf.offset + ofs_f,
                            ap=[list(Mf.ap[0]), [512, gn], [1, 64]])
            nc.vector.tensor_tensor(
                Z3[:, :, 0:64], Z3[:, :, 0:64], Meo_o, AluOpType.mult)
            nc.vector.tensor_tensor(
                Z3[:, :, 100:164], Z3[:, :, 100:164], Mes_o, AluOpType.mult)
            v["Z"] = Z
            v["Z3"] = Z3

        def sin_amp(g):
            """ACT: Sin, then amp cast (scaled by QS) into the dead U."""
            v = st[g]
            gn, U4, Mi = v["gn"], v["U4"], v["Mi"]
            S = spool.tile([P, 2 * gn * 216], F16)
            S4 = S[:].rearrange("p (b g r) -> p b g r", b=2, g=gn)
            nc.scalar.activation(
                out=S4, in_=U4, func=mybir.ActivationFunctionType.Sin,
                scale=float(QS))
            nc.scalar.activation(
                out=U4, in_=Mi[:, :, :, OFF_AMP:OFF_AMP + 216],
                func=mybir.ActivationFunctionType.Copy)
            v["S"] = S
            v["S4"] = S4

        def phase3(g):
            """DVE: amp mul, T sums, temporal products, fold."""
            v = st.pop(g)
            t0, gn = v["t0"], v["gn"]
            S, S4, U4, R, Z3 = v["S"], v["S4"], v["U4"], v["R"], v["Z3"]
            Z = v["Z"]
            nc.vector.tensor_tensor(S4, S4, U4, AluOpType.mult)
            T = tpool.tile([P, 2 * gn * 72], F16)
            T4 = T[:].rearrange("p (b g r) -> p b g r", b=2, g=gn)
            Sk = [bass.AP(
                tensor=S[:].tensor, offset=S[:].offset + 72 * k,
                ap=[list(S[:].ap[0]), [gn * 216, 2], [216, gn], [1, 72]])
                for k in range(3)]
            nc.vector.tensor_tensor(T4, Sk[0], Sk[1], AluOpType.add)
            nc.vector.tensor_tensor(T4, T4, Sk[2], AluOpType.add)
            # op1b: Z[:, :, {64,164}+0:36] = T_s * R  (T already real)
            Zf = Z[:]
            Zt = bass.AP(tensor=Zf.tensor, offset=Zf.offset + 64,
                         ap=[list(Zf.ap[0]), [RROW, gn], [100, 2], [1, 36]])
            Ts = bass.AP(tensor=T[:].tensor, offset=T[:].offset,
                         ap=[list(T[:].ap[0]), [72, gn], [36, 2], [1, 36]])
            Rt = bass.AP(tensor=R[:].tensor, offset=R[:].offset + 64,
                         ap=[list(R[:].ap[0]), [RROW, gn], [100, 2], [1, 36]])
            nc.vector.scalar_tensor_tensor(
                out=Zt, in0=Ts, scalar=float(QS), in1=Rt,
                op0=AluOpType.mult, op1=AluOpType.mult)
            # op2 temporal part with swapped o-row T factors
            To_o = bass.AP(tensor=T[:].tensor,
                           offset=T[:].offset + gn * 72 + 36,
                           ap=[list(T[:].ap[0]), [72, gn], [1, 36]])
            Ts_o = bass.AP(tensor=T[:].tensor, offset=T[:].offset + gn * 72,
                           ap=[list(T[:].ap[0]), [72, gn], [1, 36]])
            nc.vector.tensor_tensor(
                Z3[:, :, 64:100], Z3[:, :, 64:100], To_o, AluOpType.mult)
            nc.vector.tensor_tensor(
                Z3[:, :, 164:200], Z3[:, :, 164:200], Ts_o, AluOpType.mult)
            acc_v = oacc[:, t0:t0 + gn].rearrange("p (g r) -> p g r", r=1)
            nc.vector.tensor_reduce(
                out=acc_v, in_=Z3, axis=mybir.AxisListType.X, op=AluOpType.add)

        for g in range(ngroups):
            phase1(g)
            phase2(g)
            sin_amp(g)
            phase3(g)

        nc.sync.dma_start(out[:, :], oacc[:])

    _split_multi_waits(nc)
    nc.finalize()
    return nc


def _split_multi_waits(nc, limit=1):
    """walrus rejects instructions with more than one sync-wait command."""
    n = 0
    for bb in nc.main_func.blocks:
        insts = bb.instructions
        i = 0
        while i < len(insts):
            inst = insts[i]
            si = inst.sync_info
            if si is not None and len(si.on_wait) > limit:
                waits = list(si.on_wait)
                for w in waits[:-limit]:
                    nop = mybir.InstNoOp(name=f"{inst.name}-wsplit{n}",
                                         ins=[], outs=[])
                    n += 1
                    nop.engine = inst.engine
                    nop.sync_info = mybir.SyncInfo(on_wait=[w], on_update=[])
                    nc.register_instruction(nop)
                    insts.insert(i, nop)
                    i += 1
                inst.sync_info = mybir.SyncInfo(
                    on_wait=waits[-limit:], on_update=list(si.on_update))
            i += 1
    return nc


# ----------------------------------------------------------------------------
# host-side packing
# ----------------------------------------------------------------------------

def _q8(x):
    return np.clip(np.rint(np.asarray(x, np.float32) / QS), -127, 127
                   ).astype(np.int8)


def pack_tables(e_s, e_o, amp_s, frq_s, phi_s, amp_o, frq_o, phi_o):
    tbl = np.zeros((NE_PAD, ROWB), np.int8)
    ef = np.concatenate([e_s, e_o], axis=1).astype(np.float16)  # [NE, 128]
    tbl[:NE, 0:256] = ef.view(np.int8)
    for off, ts, to in ((OFF_FRQ, frq_s, frq_o), (OFF_PHI, phi_s, phi_o),
                        (OFF_AMP, amp_s, amp_o)):
        for k in range(3):
            base = off + 72 * k
            tbl[:NE, base:base + 36] = _q8(ts[k])
            tbl[:NE, base + 36:base + 72] = _q8(to[k])
    return tbl


def pack_relations(r_f, r_i):
    rp = np.empty((NR, RROW), np.float32)
    rp[:, 0:64] = 256.0 * r_f[:, 0:64]
    rp[:, 64:100] = (256.0 * QS) * r_f[:, 64:100]
    rp[:, 100:164] = 256.0 * r_i[:, 0:64]
    rp[:, 164:200] = (256.0 * QS) * r_i[:, 64:100]
    return rp.astype(np.float16)


def plan_layout(s, o):
    """Bucket-sort the full batch; deal each bucket evenly to cores.
    Returns (nt, runs, per-core slot->orig index arrays)."""
    s = np.asarray(s)
    o = np.asarray(o)
    sb = s // BANKR
    ob = o // BANKR
    bucket = sb * NBANK + ob
    order = np.argsort(bucket, kind="stable")
    bsizes = np.bincount(bucket, minlength=NBANK * NBANK)

    caps = np.zeros(NBANK * NBANK, np.int64)
    for b in range(NBANK * NBANK):
        percore = -(-bsizes[b] // NCORES)         # ceil
        caps[b] = -(-percore // P) * P if percore else 0
    nt = int(caps.sum()) // P

    # per-core slot -> original element (-1 = pad)
    slot_orig = np.full((NCORES, nt * P), -1, np.int64)
    bstart = np.zeros(NBANK * NBANK + 1, np.int64)
    np.cumsum(bsizes, out=bstart[1:])
    cstart = np.zeros(NBANK * NBANK + 1, np.int64)
    np.cumsum(caps, out=cstart[1:])
    for b in range(NBANK * NBANK):
        members = order[bstart[b]:bstart[b + 1]]
        for c in range(NCORES):
            part = members[c::NCORES]
            slot_orig[c, cstart[b]:cstart[b] + len(part)] = part

    # gather runs (identical across cores): split at group boundaries and
    # (o-side) bucket boundaries / (s-side) s-bank changes.
    runs = []
    gbound = G * P
    for side in range(2):
        cuts = {0, nt * P}
        for b in range(NBANK * NBANK):
            if caps[b] == 0:
                continue
            cuts.add(int(cstart[b]))
            cuts.add(int(cstart[b + 1]))
        # merge s-side cuts within same s_bank: recompute: only keep cut
        # if bank changes there (or group boundary)
        cl = sorted(cuts)
        segs = []
        for a, bnd in zip(cl[:-1], cl[1:]):
            if a == bnd:
                continue
            # bank of this segment
            bidx = np.searchsorted(cstart, a, side="right") - 1
            bank = (bidx // NBANK) if side == 0 else (bidx % NBANK)
            segs.append((a, bnd, int(bank)))
        # merge adjacent same-bank segments (s-side mostly)
        merged = []
        for a, bnd, bank in segs:
            if merged and merged[-1][2] == bank and merged[-1][1] == a:
                merged[-1] = (merged[-1][0], bnd, bank)
            else:
                merged.append((a, bnd, bank))
        # split at group boundaries and the SWDGE ring capacity (the
        # descriptor carveout holds 1024 descriptors per queue; one gather
        # must not exceed 1024 rows)
        for a, bnd, bank in merged:
            x = a
            while x < bnd:
                nxt = min(bnd, (x // gbound + 1) * gbound, x + 1024)
                runs.append((side, bank, int(x), int(nxt - x)))
                x = nxt
    return nt, runs, slot_orig


def _wrap16(stream):
    # idx i -> [i % 16, i // 16], replicated to all 8 GPSIMD core groups
    return np.ascontiguousarray(np.tile(stream.reshape(-1, 16).T, (8, 1)))


def pack_core_inputs(core, nt, slot_orig, s, o, r, y, m, d, rp):
    so = slot_orig[core]
    valid = so >= 0
    idx = np.where(valid, so, 0)

    sv = np.where(valid, np.asarray(s)[idx], 0).astype(np.int64)
    ov = np.where(valid, np.asarray(o)[idx], 0).astype(np.int64)
    s_rel = (sv % BANKR).astype(np.int16)
    o_rel = (ov % BANKR).astype(np.int16)
    # pads: relative id 0 of the bucket's bank (sv=0 -> rel 0, safe)
    rv = np.where(valid, np.asarray(r)[idx], 0)
    yv = np.where(valid, np.asarray(y)[idx], 0).astype(np.float16)
    mv = np.where(valid, np.asarray(m)[idx], 0).astype(np.float16)
    dv = np.where(valid, np.asarray(d)[idx], 0).astype(np.float16)

    rmat = np.ascontiguousarray(
        rp[rv].reshape(nt, P, RROW).transpose(1, 0, 2).reshape(P, nt * RROW))
    tvs = np.stack([yv, mv, dv], axis=-1)  # [nt*P, 3]
    tv = np.ascontiguousarray(
        tvs.reshape(nt, P, 3).transpose(1, 0, 2).reshape(P, nt * 3))
    return {
        "sidx": _wrap16(s_rel), "oidx": _wrap16(o_rel),
        "rmat": rmat, "tv": tv,
    }


_NC_CACHE = {}


def kernel(s, r, o, y, m, d, e_s, e_o, amp_s, frq_s, phi_s,
           amp_o, frq_o, phi_o, r_f, r_i, _trace=False):
    s = np.asarray(s); r = np.asarray(r); o = np.asarray(o)
    tbl = pack_tables(np.asarray(e_s), np.asarray(e_o), np.asarray(amp_s),
                      np.asarray(frq_s), np.asarray(phi_s), np.asarray(amp_o),
                      np.asarray(frq_o), np.asarray(phi_o))
    rp = pack_relations(np.asarray(r_f), np.asarray(r_i))

    nt, runs, slot_orig = plan_layout(s, o)
    key = (nt, tuple(runs))
    if key not in _NC_CACHE:
        _NC_CACHE.clear()
        _NC_CACHE[key] = build_nc(nt, runs)
    nc = _NC_CACHE[key]

    in_maps = []
    for c in range(NCORES):
        im = pack_core_inputs(c, nt, slot_orig, s, o, r,
                              np.asarray(y), np.asarray(m), np.asarray(d), rp)
        im["tbl"] = tbl
        in_maps.append(im)

    res = run_bass_kernel_spmd(nc, in_maps, list(range(NCORES)), trace=_trace)
    full = np.empty(B, np.float32)
    for c in range(NCORES):
        vals = np.asarray(res.results[c]["out"]).T.reshape(-1)  # slot order
        so = slot_orig[c]
        vld = so >= 0
        full[so[vld]] = vals[vld]
    full /= KOUT
    if _trace:
        return full, res
    return full
